# revision 1
# baseline (speedup 1.0000x reference)
"""Trainium2 Bass kernel for the CustomLSTM problem.

Problem: B=2048, T=256, I=5, H=50, O=1 LSTM; y = fc(h_T).

Structure (data-parallel 8 cores x 256 batch; per core, CHAINS=3
independent column-group scans whose serial per-step dependency chains
interleave on the engines):

  - The recurrence is latency-bound: the per-step serial path
      mm(gates) -> Sigmoid -> [ts, P on DVE] -> PE-reduce -> Tanh ->
      h-mul -> mm
    carries ~1.16us of fixed inter-engine handoff latency (pipeline
    drains + semaphore hops), so wall ~= T * path.  Three narrower
    chains (86/85/85 batch cols) shrink the variable per-column part of
    every op on the path while the ACT engine still fits all chains'
    Sigmoid+Tanh work per step (~76% busy).
  - x is PRELOADED to SBUF once as X_all [5, T*BL] fp16 (~131KB on 5
    partitions); a DVE copy drops each step's slice into the rhs ring
    rows 64:69 (GPSIMD/Pool copies measure far slower than modeled).  No per-step DMA (per-step x DMAs keep the
    single HWDGE queue 625ns/transfer busy and cap CHAINS at 2).
  - rhs ring layout [h(0:50); pad; x(64:69); 1(69)], K=70: one matmul
    per gate-pack carries the recurrent part, the input projection AND
    all biases (ones row).  x sits at partition 64, not 50, because
    engine outputs must start at partition 0/32/64/96 (DMA was exempt;
    a Pool copy is not).
  - Gate layout per pack: [f/o at rows 0:50, i/2c at rows 64:114]; the
    c-gate weights are pre-doubled so ONE Sigmoid over [128, 2BW] yields
    F, I, O and U = sig(2 g_c), with tanh(g_c) = 2U - 1 via one
    tensor_scalar.  c' = P[0:50] + P[64:114] (P = [F;I]*[c;c~]) is done
    on the PE with a stacked-identity lhsT: a DVE tensor_tensor add of
    two SBUF operands at different base partitions is ILLEGAL
    (neuronxcc NCC_IBIR297 requires equal base partitions).
  - Tanh reads c' straight from PSUM; the SBUF copy of the c state is
    emitted after the h-multiply so the in-order DVE keeps h off the
    critical path.
  - NO scheduler phase hints: hint-pinned modulo schedules tuned to the
    cost model measured 723us on hardware (real per-op costs shift and
    the pinned FIFO order cascades into stalls); the scheduler's greedy
    readiness order measures 612us for the same instruction set.
  - fp16 SBUF, fp32 PSUM accumulation; end-to-end error vs the fp32
    reference ~9e-4 (validated in CoreSim and on hardware).
  - _build_bass(reps=N) wraps the time loop in a hardware For_i for
    device-side differential timing (see timing_kernel.py).
"""

import numpy as np

B, T, IN, H, OUT = 2048, 256, 5, 50, 1
NCORES = 8
BL = B // NCORES  # 256 batch per core
KX = IN + 1  # 6: [x; 1]
KR = 70  # rhs rows: h(0:50), pad(50:64), x(64:69), ones(69)
XOFF = 64
H2 = 64  # partition offset of the second gate in each pair
M = 128  # matmul output partitions (f/o at 0:50, i/c at 64:114)
NR = 2  # h ring buffers
CHAINS = 3
SB_DT = "f16"

_widths = [BL // CHAINS + (1 if i < BL % CHAINS else 0) for i in range(CHAINS)]
_starts = [sum(_widths[:i]) for i in range(CHAINS)]

# scheduler phase hints (virtual ns; order-only, see module docstring)
HINT_PERIOD_NS = 2300.0
HINT_STAG_NS = 766.0


def _np_dt():
    return np.float16 if SB_DT == "f16" else np.float32


def _build_weights(inp, np_dt):
    Whf, Whi, Whc, Who = inp["Whf"], inp["Whi"], inp["Whc"], inp["Who"]
    Wxf, Wxi, Wxc, Wxo = inp["Wxf"], inp["Wxi"], inp["Wxc"], inp["Wxo"]
    b_f = inp["bxf"] + inp["bhf"] + inp["bf"]
    b_i = inp["bxi"] + inp["bhi"] + inp["bi"]
    b_c = inp["bxc"] + inp["bhc"] + inp["bc"]
    b_o = inp["bxo"] + inp["bho"] + inp["bo"]

    def pack(Wh_a, Wx_a, b_a, Wh_b, Wx_b, b_b, scale_b=1.0):
        W = np.zeros((KR, M), dtype=np.float64)
        W[0:H, 0:H] = Wh_a.T
        W[XOFF : XOFF + IN, 0:H] = Wx_a.T
        W[XOFF + IN, 0:H] = b_a
        W[0:H, H2 : H2 + H] = scale_b * Wh_b.T
        W[XOFF : XOFF + IN, H2 : H2 + H] = scale_b * Wx_b.T
        W[XOFF + IN, H2 : H2 + H] = scale_b * b_b
        return W

    W_fi = pack(Whf, Wxf, b_f, Whi, Wxi, b_i)
    W_co = pack(Who, Wxo, b_o, Whc, Wxc, b_c, scale_b=2.0)

    Ired = np.zeros((M, H2), dtype=np.float64)
    Ired[0:H, 0:H] = np.eye(H)
    Ired[H2 : H2 + H, 0:H] = np.eye(H)

    W_fc = np.zeros((KR, OUT), dtype=np.float64)
    W_fc[0:H, 0] = inp["Wfc"][0]
    W_fc[XOFF + IN, 0] = inp["bfc"][0]

    return tuple(
        np.ascontiguousarray(w, dtype=np_dt)
        for w in (W_fi, W_co, W_fc, Ired)
    )


def _build_bass(T=T, chains=CHAINS, reps=None):
    import concourse.mybir as mybir
    from concourse import bacc, tile

    f32 = mybir.dt.float32
    sb = mybir.dt.float16 if SB_DT == "f16" else mybir.dt.float32
    AF = mybir.ActivationFunctionType
    ALU = mybir.AluOpType

    nc = bacc.Bacc(None)

    x_d = nc.dram_tensor("x_all", [IN, T * BL], sb, kind="ExternalInput")
    wfi_d = nc.dram_tensor("w_fi", [KR, M], sb, kind="ExternalInput")
    wco_d = nc.dram_tensor("w_co", [KR, M], sb, kind="ExternalInput")
    wfc_d = nc.dram_tensor("w_fc", [KR, OUT], sb, kind="ExternalInput")
    ired_d = nc.dram_tensor("i_red", [M, H2], sb, kind="ExternalInput")
    out_d = nc.dram_tensor("out", [OUT, BL], f32, kind="ExternalOutput")

    with tile.TileContext(nc) as tc:
        with (
            tc.tile_pool(name="const", bufs=1) as cpool,
            tc.tile_pool(name="state", bufs=1) as spool,
            tc.tile_pool(name="work", bufs=2) as wpool,
            tc.tile_pool(name="psum", bufs=1, space="PSUM") as pg_pool,
            tc.tile_pool(name="psum_c", bufs=1, space="PSUM") as pc_pool,
            tc.tile_pool(name="psum_fc", bufs=1, space="PSUM") as pfc_pool,
        ):
            X_all = cpool.tile([IN, T * BL], sb, tag="x_all")
            wfi = cpool.tile([KR, M], sb, tag="wfi")
            wco = cpool.tile([KR, M], sb, tag="wco")
            wfc = cpool.tile([KR, OUT], sb, tag="wfc")
            ired = cpool.tile([M, H2], sb, tag="ired")
            for t_sb, t_d in (
                (X_all, x_d), (wfi, wfi_d), (wco, wco_d),
                (wfc, wfc_d), (ired, ired_d),
            ):
                nc.sync.dma_start(t_sb[:], t_d[:])

            # per-chain rhs ring: rows 0:50 h, 64:69 x, 69 ones
            R = [
                [
                    spool.tile([KR, _widths[c]], sb, tag=f"R{c}_{i}",
                               name=f"R{c}_{i}")
                    for i in range(NR)
                ]
                for c in range(chains)
            ]
            # per-chain state: rows 0:50 = c, rows 64:114 = c~ scratch
            S = [
                spool.tile([M, _widths[c]], sb, tag=f"S{c}", name=f"S{c}")
                for c in range(chains)
            ]
            # per-chain P scratch: rows 0:50 = F*c, rows 64:114 = I*c~
            P = [
                spool.tile([M, _widths[c]], sb, tag=f"P{c}", name=f"P{c}")
                for c in range(chains)
            ]
            for c in range(chains):
                nc.vector.memset(S[c][:], 0.0)
                for i in range(NR):
                    nc.vector.memset(R[c][i][:], 0.0)
                    # ones row (x rows 64:69 get overwritten each step)
                    nc.vector.memset(R[c][i][XOFF:KR, :], 1.0)

            import contextlib
            loop_ctx = (
                tc.For_i(0, reps) if reps is not None and reps > 1
                else contextlib.nullcontext()
            )
            with loop_ctx:
                for t in range(T):
                    for c in range(chains):
                        c0 = t * BL + _starts[c]
                        BW = _widths[c]
                        r = R[c][t % NR]

                        # x into ring rows 64:69 from SBUF-resident X_all, on
                        # the idle Pool engine (hinted one step early; base
                        # partition 64 is a legal engine-output start)
                        hint_x = tc.tile_wait_until(
                            0.0
                        )
                        hint_x.__enter__()
                        nc.vector.tensor_copy(
                            r[XOFF : XOFF + IN, :], X_all[:, c0 : c0 + BW]
                        )
                        hint_x.__exit__(None, None, None)

                        hint = tc.tile_wait_until(
                            0.0
                        )
                        hint.__enter__()

                        pg = pg_pool.tile(
                            [M, 2 * BW], f32, tag=f"pg{c}", name=f"pg{c}"
                        )
                        nc.tensor.matmul(
                            pg[:, 0:BW], wfi[:], r[:], start=True, stop=True
                        )
                        nc.tensor.matmul(
                            pg[:, BW : 2 * BW], wco[:], r[:], start=True, stop=True
                        )

                        # G: [F; I] cols 0:BW, [O; U] cols BW:2BW
                        G = wpool.tile([M, 2 * BW], sb, tag=f"G{c}", name=f"G{c}")
                        nc.scalar.activation(G[:], pg[:], AF.Sigmoid)

                        # c~ = 2U - 1 into S rows 64:114
                        nc.vector.tensor_scalar(
                            out=S[c][H2 : H2 + H, :],
                            in0=G[H2 : H2 + H, BW : 2 * BW],
                            scalar1=2.0,
                            scalar2=1.0,
                            op0=ALU.mult,
                            op1=ALU.subtract,
                        )
                        # P = [F; I] * [c; c~]   (pad rows: G*0 = 0)
                        nc.vector.tensor_mul(P[c][:], G[:, 0:BW], S[c][:])
                        # c' = P[0:50] + P[64:114] via PE reduction (compute
                        # engines cannot add across partition bases)
                        pc = pc_pool.tile([H2, BW], f32, tag=f"pc{c}",
                                          name=f"pc{c}")
                        nc.tensor.matmul(
                            pc[:], ired[:], P[c][:], start=True, stop=True
                        )

                        # V = tanh(c')
                        V = wpool.tile([H, BW], sb, tag=f"V{c}", name=f"V{c}")
                        nc.scalar.activation(V[:], pc[0:H, :], AF.Tanh)

                        # h = O * V into the next ring slot (before the
                        # c'-copy so the in-order DVE does not stall h)
                        rn = R[c][(t + 1) % NR]
                        nc.vector.tensor_mul(
                            rn[0:H, :], G[0:H, BW : 2 * BW], V[:]
                        )
                        # c (sbuf) = c' — off the critical path
                        nc.vector.tensor_copy(S[c][0:H, :], pc[0:H, :])
                        hint.__exit__(None, None, None)

            # final fc per chain: out = Wfc @ h_T + bfc (bias via the ones
            # row of X_all through wfc_x)
            for c in range(chains):
                BW = _widths[c]
                pfc = pfc_pool.tile([OUT, BW], f32, tag="pfc", name=f"pfc{c}")
                nc.tensor.matmul(
                    pfc[:], wfc[:], R[c][T % NR][:], start=True, stop=True
                )
                out_sb = wpool.tile([OUT, BW], f32, tag=f"osb{c}",
                                    name=f"osb{c}")
                nc.scalar.copy(out=out_sb[:], in_=pfc[:])
                nc.sync.dma_start(
                    out_d[:, _starts[c] : _starts[c] + _widths[c]], out_sb[:]
                )

    nc.compile()
    return nc


_NC_CACHE = None


def _get_nc():
    global _NC_CACHE
    if _NC_CACHE is None:
        _NC_CACHE = _build_bass()
    return _NC_CACHE


def _in_maps(inp):
    np_dt = _np_dt()
    W_fi, W_co, W_fc, Ired = _build_weights(inp, np_dt)
    in_maps = []
    for k in range(NCORES):
        xs = inp["x"][k * BL : (k + 1) * BL]  # [BL, T, IN]
        xT = np.transpose(xs, (2, 1, 0)).reshape(IN, T * BL)
        in_maps.append(
            {
                "x_all": np.ascontiguousarray(xT, dtype=np_dt),
                "w_fi": W_fi,
                "w_co": W_co,
                "w_fc": W_fc,
                "i_red": Ired,
            }
        )
    return in_maps


def kernel(**inputs):
    from concourse.bass_utils import run_bass_kernel_spmd

    inp = {k: np.asarray(v, dtype=np.float32) for k, v in inputs.items()}
    nc = _get_nc()
    res = run_bass_kernel_spmd(nc, _in_maps(inp), list(range(NCORES)))
    outs = [res.results[k]["out"].reshape(BL, OUT) for k in range(NCORES)]
    return np.concatenate(outs, axis=0).astype(np.float32)

